# revision 1
# baseline (speedup 1.0000x reference)
"""MultiHeadLatentAttention (MLA) Trainium2 Bass kernel.

Problem: B=2, S=2048, D=2048, H=16 heads, d_nope=128, d_rope=64, d_head=128,
q_latent=768, kv_latent=512. Causal attention, rmsnorm'd latents, half-dim RoPE.

Sharding (8 cores): core c handles batch b=c//4 and head group g=c%4 (4 heads).
The small latent down-projections are replicated within each batch group;
W_uq/W_qr/W_uk/W_kr/W_uv are column-sharded by head; W_o row-sharded; the
4 partial outputs per batch are summed on the host.

Device dataflow (everything in "transposed" layout, features on partitions,
sequence on the free dim, so every matmul uses weights as-stored for lhsT and
all moving operands have free dim 512):
  P0: q_latT/kv_latT = W_d*^T @ x^T, rmsnorm via ones-matmul sumsq +
      exp(-0.5*ln(ms)) + gpsimd partition_broadcast; latents round-trip DRAM.
  P1: kT (nope + rope pairs) and v (natural layout) up-projections.
  P2: per 512-wide q-block: q up-proj on demand, scores^T = k^T(tile)^T q^T
      with additive causal mask applied via an identity matmul of a
      precomputed mask; exp batched over [128,1024] two-bank PSUM tiles on
      ACT; softmax denominator accumulated on the DVE and finished with a
      gpsimd partition_all_reduce (broadcast sum), 1/den = exp(-ln(den));
      PV matmuls (staggered one exp-pair behind the scores matmuls) give
      out^T; then y^T += W_o^T out^T, deferred one q-block for overlap.
      Projection chains alternate between two PSUM pools for 4-deep
      chain pipelining within the 8-bank budget.

All matmuls run as float32r (FP22 multiply, fp32 accumulate, 1 cycle/row with
512-wide moving operands) — measured end-to-end relative error vs the fp32
reference is ~4e-4.

MLA_ALLGATHER=1 switches to an S-sharded down-projection with a device
AllGather of the latents within each 4-core batch group (saves ~110us of
replicated down-projection matmuls per core, but the cost model prices the
10.5MB gather at ~250us, so it is off by default).
"""
import math
import os
from contextlib import ExitStack

import numpy as np

import concourse.bass as bass
import concourse.bass_isa as bass_isa
import concourse.bacc as bacc
import concourse.mybir as mybir
import concourse.tile as tile
from concourse.bass_utils import run_bass_kernel_spmd

F32 = mybir.dt.float32
F32R = mybir.dt.float32r
AF = mybir.ActivationFunctionType

B, S_FULL, D = 2, 2048, 2048
H, DN, DR, DH = 16, 128, 64, 128
QL, KVL = 768, 512
EPS = 1e-6
SCALE = 1.0 / math.sqrt(DH)
MASK_NEG = -1e6
NCORES = 8
NKT = D // 128          # 16 contraction tiles over D
NLQ = QL // 128         # 6
NLKV = KVL // 128       # 4
NDT = D // 128          # 16 output D tiles


def _rope_apply(nc, pool, ps, c4s, s4s, out_ap):
    """Half-dim rope on a pair tile [128, 512] (h_even x1|x2 | h_odd x1|x2).

    out = ps * c4 + shuf(ps) * s4,  shuf swaps the 32-blocks within each 64.
    ps is PSUM; out_ap is SBUF.
    """
    shuf = pool.tile([128, 512], F32, tag="rope_shuf")
    nc.vector.tensor_copy(shuf[0:32, :], ps[32:64, :])
    nc.vector.tensor_copy(shuf[32:64, :], ps[0:32, :])
    nc.vector.tensor_copy(shuf[64:96, :], ps[96:128, :])
    nc.vector.tensor_copy(shuf[96:128, :], ps[64:96, :])
    t1 = pool.tile([128, 512], F32, tag="rope_t1")
    nc.vector.tensor_mul(t1[:], ps[:], c4s)
    nc.vector.tensor_mul(out_ap, shuf[:], s4s)
    nc.vector.tensor_add(out_ap, out_ap, t1[:])


PHASE_MARKS = {}


def build_nc(S=S_FULL, allgather=None):
    assert S % 512 == 0
    n_sb = S // 512
    n_st = S // 128
    if allgather is None:
        allgather = bool(int(os.environ.get("MLA_ALLGATHER", "0")))
    PHASE_MARKS.clear()

    nc = bacc.Bacc("TRN2", target_bir_lowering=False, debug=False,
                   num_devices=NCORES)

    x_cols = 512 if allgather else S
    xT_d = nc.dram_tensor("xT", [D, x_cols], F32R, kind="ExternalInput")
    wdq_d = nc.dram_tensor("W_dq", [D, QL], F32R, kind="ExternalInput")
    wdkv_d = nc.dram_tensor("W_dkv", [D, KVL], F32R, kind="ExternalInput")
    wuq_d = nc.dram_tensor("Wuq", [QL, 512], F32R, kind="ExternalInput")
    wqr_d = nc.dram_tensor("Wqr", [QL, 256], F32R, kind="ExternalInput")
    wuk_d = nc.dram_tensor("Wuk", [KVL, 512], F32R, kind="ExternalInput")
    wkr_d = nc.dram_tensor("Wkr", [KVL, 256], F32R, kind="ExternalInput")
    wuv_d = nc.dram_tensor("Wuv", [KVL, 512], F32R, kind="ExternalInput")
    wo_d = nc.dram_tensor("Wo", [512, D], F32R, kind="ExternalInput")
    c4_d = nc.dram_tensor("c4", [128, S], F32, kind="ExternalInput")
    s4_d = nc.dram_tensor("s4", [128, S], F32, kind="ExternalInput")
    mask_d = nc.dram_tensor("mask_big", [128, 896], F32R, kind="ExternalInput")
    id_d = nc.dram_tensor("ident", [128, 128], F32R, kind="ExternalInput")
    ones_d = nc.dram_tensor("ones_col", [128, 1], F32R, kind="ExternalInput")
    yT_d = nc.dram_tensor("yT", [D, S], F32, kind="ExternalOutput")

    def col3(dram_ap, p=128):
        # [R, C] dram slice -> [128, R//128, C] tiled AP
        return dram_ap.rearrange("(t p) c -> p t c", p=p)

    with tile.TileContext(nc) as tc:
        with (
            tc.tile_pool(name="const", bufs=1) as constp,
            tc.tile_pool(name="ps_mm", bufs=2, space="PSUM") as ps_mm,
            tc.tile_pool(name="ps_o", bufs=2, space="PSUM") as ps_op,
            tc.tile_pool(name="dram", bufs=1, space="DRAM") as dramp,
        ):
            if allgather:
                lat_in = dramp.tile([QL + KVL, 512], F32R)
                lat_out = dramp.tile([n_sb * (QL + KVL), 512], F32R)

                def qlat_src(sb):
                    return lat_out[sb * 1280:sb * 1280 + QL, :]

                def kvlat_src(sb):
                    return lat_out[sb * 1280 + QL:(sb + 1) * 1280, :]
            else:
                qlat_ds = [
                    dramp.tile([QL, 512], F32R, tag=f"qlat{i}", name=f"qlat{i}")
                    for i in range(n_sb)
                ]
                kvlat_ds = [
                    dramp.tile([KVL, 512], F32R, tag=f"kvlat{i}",
                               name=f"kvlat{i}")
                    for i in range(n_sb)
                ]

                def qlat_src(sb):
                    return qlat_ds[sb][:]

                def kvlat_src(sb):
                    return kvlat_ds[sb][:]
            def alt_ps(i):
                if i % 2 == 0:
                    return ps_mm.tile([128, 512], F32, tag="mm", name="ps")
                return ps_op.tile([128, 512], F32, tag="pv", name="ps")

            mask_t = constp.tile([128, 896], F32R)
            id_t = constp.tile([128, 128], F32R)
            ones_t = constp.tile([128, 1], F32R)
            eps_t = constp.tile([1, 1], F32)
            # ---------------- P0: down-projections + rmsnorm ----------------
            PHASE_MARKS["P0"] = nc.next_id()
            p1_stack = ExitStack()
            p1lat = p1_stack.enter_context(tc.tile_pool(name="p1lat", bufs=2))
            kvl_tiles = {}
            with (
                tc.tile_pool(name="p0w", bufs=1) as p0w,
                tc.tile_pool(name="p0x", bufs=4) as p0x,
                tc.tile_pool(name="p0latq", bufs=2) as p0latq,
                tc.tile_pool(name="p0latkv", bufs=1) as p0latkv,
                tc.tile_pool(name="p0tmp", bufs=1) as p0tmp,
                tc.tile_pool(name="p0rsb", bufs=2) as p0rsb,
                tc.tile_pool(name="ps_den", bufs=1, space="PSUM") as ps_denp,
                tc.tile_pool(name="ps_p0", bufs=2, space="PSUM") as ps_p0,
            ):
                def alt3_ps(i):
                    if i % 3 == 2:
                        return ps_p0.tile([128, 512], F32, tag="p0", name="ps")
                    return alt_ps(i % 3)
                wdq_t = p0w.tile([128, NKT, QL], F32R)
                wdkv_t = p0w.tile([128, NKT, KVL], F32R)

                def load_wdq(lt):
                    nc.sync.dma_start(
                        wdq_t[:, :, 128 * lt:128 * (lt + 1)],
                        col3(wdq_d[:, 128 * lt:128 * (lt + 1)]),
                    )

                for g4 in range(4):
                    nc.sync.dma_start(
                        wdq_t[:, 4 * g4:4 * (g4 + 1), 0:128],
                        col3(wdq_d[:, 0:128])[:, 4 * g4:4 * (g4 + 1), :],
                    )
                n_local = 1 if allgather else n_sb
                for sb in range(n_local):
                    cs = slice(512 * sb, 512 * (sb + 1))
                    xh = [
                        p0x.tile([128, 8, 512], F32R, tag="xsb", name=f"xsb{i}")
                        for i in range(2)
                    ]
                    for kt in range(NKT):
                        nc.sync.dma_start(
                            xh[kt // 8][:, kt % 8, :],
                            col3(xT_d[:, cs])[:, kt, :],
                        )
                    if sb == 0:
                        nc.sync.dma_start(ones_t[:], ones_d[:])
                        nc.vector.memset(eps_t[:], EPS)
                        for lt in range(1, NLQ):
                            load_wdq(lt)
                        for lt in range(NLKV):
                            nc.sync.dma_start(
                                wdkv_t[:, :, 128 * lt:128 * (lt + 1)],
                                col3(wdkv_d[:, 128 * lt:128 * (lt + 1)]),
                            )
                    for latname, w_t, nl in (("q", wdq_t, NLQ), ("kv", wdkv_t, NLKV)):
                        raw = (p0latq if latname == "q" else p0latkv).tile(
                            [128, nl, 512], F32R, tag=f"raw{latname}",
                            name=f"raw{latname}")
                        ps_ss = ps_denp.tile([1, 512], F32, tag="den")
                        for lt in range(nl):
                            ps = alt3_ps(lt)
                            for kt in range(NKT):
                                nc.tensor.matmul(
                                    ps[:],
                                    w_t[:, kt, 128 * lt:128 * (lt + 1)],
                                    xh[kt // 8][:, kt % 8, :],
                                    start=(kt == 0), stop=(kt == NKT - 1),
                                )
                            nc.scalar.copy(raw[:, lt, :], ps[:])
                            sq = p0tmp.tile([128, 512], F32R, tag="sq")
                            nc.vector.tensor_mul(sq[:], raw[:, lt, :], raw[:, lt, :])
                            nc.tensor.matmul(
                                ps_ss[:], ones_t[:], sq[:],
                                start=(lt == 0), stop=(lt == nl - 1),
                            )
                        lrow = p0tmp.tile([1, 512], F32, tag="lrow")
                        nc.scalar.activation(
                            lrow[:], ps_ss[:], AF.Ln, scale=1.0 / (128 * nl),
                            bias=eps_t[:],
                        )
                        rrow = p0tmp.tile([1, 512], F32R, tag="rrow")
                        nc.scalar.activation(rrow[:], lrow[:], AF.Exp, scale=-0.5)
                        rsb = p0rsb.tile([128, 512], F32R, tag="rsb")
                        nc.gpsimd.partition_broadcast(rsb[:], rrow[:])
                        for lt in range(nl):
                            nc.vector.tensor_mul(raw[:, lt, :], raw[:, lt, :], rsb[:])
                        if allgather:
                            lat_ap = (lat_in[0:QL, :] if latname == "q"
                                      else lat_in[QL:QL + KVL, :])
                        else:
                            lat_ap = (qlat_ds if latname == "q"
                                      else kvlat_ds)[sb][:]
                        nc.sync.dma_start(col3(lat_ap), raw[:])
                    if sb == 0 and not allgather:
                        kvl0 = p1lat.tile([128, NLKV, 512], F32R, tag="kvl",
                                          name="kvl0")
                        nc.sync.dma_start(kvl0[:], col3(kvlat_src(0)))
                        kvl_tiles[0] = kvl0

            nc.sync.dma_start(mask_t[:], mask_d[:])
            nc.sync.dma_start(id_t[:], id_d[:])
            if allgather:
                nc.gpsimd.collective_compute(
                    "AllGather",
                    mybir.AluOpType.bypass,
                    replica_groups=[[0, 1, 2, 3], [4, 5, 6, 7]],
                    ins=[lat_in[:]],
                    outs=[lat_out[:]],
                )

            # ---------------- P1: k/v up-projections ----------------
            PHASE_MARKS["P1"] = nc.next_id()
            persist_stack = ExitStack()
            persist = persist_stack.enter_context(
                tc.tile_pool(name="persist", bufs=1)
            )
            kTn_t = persist.tile([128, 4, S], F32R)    # nope k^T per head
            kTr_t = persist.tile([128, 2, S], F32R)    # rope k^T per pair
            v_t = persist.tile([128, n_st, 512], F32R)  # v natural
            p2w_stack = ExitStack()
            p2w = p2w_stack.enter_context(tc.tile_pool(name="p2w", bufs=1))
            p2lat = p2w_stack.enter_context(tc.tile_pool(name="p2lat", bufs=1))
            wuq_t = p2w.tile([128, NLQ, 512], F32R)
            wqr_t = p2w.tile([128, NLQ, 256], F32R)
            qlat0 = p2lat.tile([128, NLQ, 512], F32R, tag="qlat", name="qlat0")
            with (
                tc.tile_pool(name="p1w", bufs=1) as p1w,
                tc.tile_pool(name="p1tmp", bufs=2) as p1tmp,
            ):
                wuk_t = p1w.tile([128, NLKV, 512], F32R)
                wkr_t = p1w.tile([128, NLKV, 256], F32R)
                wuv_t = p1w.tile([128, NLKV, 512], F32R)
                nc.sync.dma_start(wuk_t[:], col3(wuk_d[:]))
                for sb in range(n_sb):
                    cs = slice(512 * sb, 512 * (sb + 1))
                    if sb in kvl_tiles:
                        kvl = kvl_tiles[sb]
                    else:
                        kvl = p1lat.tile([128, NLKV, 512], F32R, tag="kvl",
                                         name=f"kvl{sb}")
                        nc.sync.dma_start(kvl[:], col3(kvlat_src(sb)))
                    if sb == 0:
                        nc.sync.dma_start(wkr_t[:], col3(wkr_d[:]))
                        nc.sync.dma_start(wuv_t[:], col3(wuv_d[:]))
                    c4s = p1tmp.tile([128, 512], F32, tag="c4")
                    s4s = p1tmp.tile([128, 512], F32, tag="s4")
                    nc.sync.dma_start(c4s[:], c4_d[:, cs])
                    nc.sync.dma_start(s4s[:], s4_d[:, cs])
                    for h in range(4):
                        ps = alt_ps(h)
                        for kl in range(NLKV):
                            nc.tensor.matmul(
                                ps[:], wuk_t[:, kl, 128 * h:128 * (h + 1)],
                                kvl[:, kl, :],
                                start=(kl == 0), stop=(kl == NLKV - 1),
                            )
                        nc.scalar.copy(kTn_t[:, h, cs], ps[:])
                    for pr in range(2):
                        ps = alt_ps(pr)
                        for kl in range(NLKV):
                            nc.tensor.matmul(
                                ps[:], wkr_t[:, kl, 128 * pr:128 * (pr + 1)],
                                kvl[:, kl, :],
                                start=(kl == 0), stop=(kl == NLKV - 1),
                            )
                        _rope_apply(nc, p1tmp, ps, c4s[:], s4s[:], kTr_t[:, pr, cs])
                    for stl in range(4):
                        st = 4 * sb + stl
                        ps = alt_ps(stl)
                        for kl in range(NLKV):
                            nc.tensor.matmul(
                                ps[:],
                                kvl[:, kl, 128 * stl:128 * (stl + 1)],
                                wuv_t[:, kl, :],
                                start=(kl == 0), stop=(kl == NLKV - 1),
                            )
                        nc.scalar.copy(v_t[:, st, :], ps[:])
                    if sb == min(1, n_sb - 1):
                        nc.sync.dma_start(wuq_t[:], col3(wuq_d[:]))
                        nc.sync.dma_start(wqr_t[:], col3(wqr_d[:]))
                        nc.sync.dma_start(qlat0[:], col3(qlat_src(0)))

            # ---------------- P2: attention + W_o ----------------
            PHASE_MARKS["P2"] = nc.next_id()
            with (
                tc.tile_pool(name="p2wo", bufs=3) as p2wo,
                tc.tile_pool(name="p2q", bufs=4) as p2q,
                tc.tile_pool(name="p2qr", bufs=2) as p2qr,
                tc.tile_pool(name="p2exp", bufs=2) as p2exp,
                tc.tile_pool(name="ps_s", bufs=2, space="PSUM") as ps_sp,
                tc.tile_pool(name="p2acc", bufs=2) as p2acc,
                tc.tile_pool(name="p2acc1", bufs=1) as p2acc1,
                tc.tile_pool(name="p2out", bufs=9) as p2out,
                tc.tile_pool(name="p2tmp", bufs=2) as p2tmp,
                tc.tile_pool(name="p2y", bufs=2) as p2y,
            ):
                def emit_wo(out_tiles, cs):
                    for dt in range(NDT):
                        woc = p2wo.tile([128, 4, 128], F32R, tag="wo", name="woc")
                        nc.sync.dma_start(
                            woc[:], col3(wo_d[:, 128 * dt:128 * (dt + 1)])
                        )
                        ps_y = ps_mm.tile([128, 512], F32, tag="mm", name="ps")
                        for h in range(4):
                            nc.tensor.matmul(
                                ps_y[:], woc[:, h, :],
                                out_tiles[h][:],
                                start=(h == 0), stop=(h == 3),
                            )
                        ystage = p2y.tile([128, 512], F32, tag="y")
                        nc.vector.tensor_copy(ystage[:], ps_y[:])
                        nc.sync.dma_start(
                            yT_d[128 * dt:128 * (dt + 1), cs], ystage[:]
                        )

                prev_out = None
                prev_cs = None
                for qb in range(n_sb):
                    cs = slice(512 * qb, 512 * (qb + 1))
                    if qb == 0:
                        qlat = qlat0
                    else:
                        qlat = p2lat.tile([128, NLQ, 512], F32R, tag="qlat",
                                          name=f"qlat{qb}")
                        nc.sync.dma_start(qlat[:], col3(qlat_src(qb)))
                    c4s = p2tmp.tile([128, 512], F32, tag="c4")
                    s4s = p2tmp.tile([128, 512], F32, tag="s4")
                    nc.sync.dma_start(c4s[:], c4_d[:, cs])
                    nc.sync.dma_start(s4s[:], s4_d[:, cs])
                    qr_tiles = []
                    for pr in range(2):
                        ps = alt_ps(pr)
                        for ql in range(NLQ):
                            nc.tensor.matmul(
                                ps[:], wqr_t[:, ql, 128 * pr:128 * (pr + 1)],
                                qlat[:, ql, :],
                                start=(ql == 0), stop=(ql == NLQ - 1),
                            )
                        qr = p2qr.tile([128, 512], F32R, tag="qr")
                        _rope_apply(nc, p2tmp, ps, c4s[:], s4s[:], qr[:])
                        qr_tiles.append(qr)
                    qn_tiles = []
                    for h in range(4):
                        ps = alt_ps(h)
                        for ql in range(NLQ):
                            nc.tensor.matmul(
                                ps[:], wuq_t[:, ql, 128 * h:128 * (h + 1)],
                                qlat[:, ql, :],
                                start=(ql == 0), stop=(ql == NLQ - 1),
                            )
                        qn = p2q.tile([128, 512], F32R, tag="qn", name=f"qn{h}")
                        nc.vector.tensor_copy(qn[:], ps[:])
                        qn_tiles.append(qn)
                    out_tiles = []
                    for h in range(4):
                        qn = qn_tiles[h]
                        qr = qr_tiles[h // 2]
                        pb = 64 * (h % 2)
                        nkt = 4 * (qb + 1)
                        ps_o = ps_op.tile([128, 512], F32, tag="pv")
                        # softmax denominator: DVE accumulates exp half 0,
                        # GPSIMD half 1; exp batches two score tiles per ACT op.
                        dacc = p2acc.tile([128, 512], F32R, tag="dacc")

                        def emit_pv(exp_pair, pk, npair, ps_o=ps_o, h=h):
                            for j in (0, 1):
                                kt = 2 * pk + j
                                nc.tensor.matmul(
                                    ps_o[:],
                                    v_t[:, kt, 128 * h:128 * (h + 1)],
                                    exp_pair[:, 512 * j:512 * (j + 1)],
                                    start=(kt == 0), stop=(kt == 2 * npair - 1),
                                )

                        npair = nkt // 2
                        pend = []   # (exp pair tile, pk) one pair behind
                        for pk in range(npair):
                            ps_s = ps_sp.tile([128, 1024], F32, tag="scores")
                            for j in (0, 1):
                                kt = 2 * pk + j
                                ks = slice(128 * kt, 128 * (kt + 1))
                                delta = 128 * kt - 512 * qb
                                diag = delta >= 0
                                half = ps_s[:, 512 * j:512 * (j + 1)]
                                nc.tensor.matmul(
                                    half, kTn_t[:, h, ks], qn[:],
                                    start=True, stop=False,
                                )
                                nc.tensor.matmul(
                                    half,
                                    kTr_t[pb:pb + 64, h // 2, ks],
                                    qr[pb:pb + 64, :],
                                    start=False, stop=not diag,
                                )
                                if diag:
                                    nc.tensor.matmul(
                                        half, id_t[:],
                                        mask_t[:, 384 - delta:896 - delta],
                                        start=False, stop=True,
                                    )
                            exp_t = p2exp.tile([128, 1024], F32R, tag="exp")
                            nc.scalar.activation(
                                exp_t[:], ps_s[:], AF.Exp, scale=SCALE
                            )
                            if pk == 0:
                                nc.vector.tensor_copy(dacc[:], exp_t[:, 0:512])
                            else:
                                nc.vector.tensor_add(
                                    dacc[:], dacc[:], exp_t[:, 0:512])
                            nc.vector.tensor_add(
                                dacc[:], dacc[:], exp_t[:, 512:1024])
                            pend.append((exp_t, pk))
                            if len(pend) > 1:
                                emit_pv(*pend.pop(0), npair)
                        for e in pend:
                            emit_pv(*e, npair)
                        red = p2acc1.tile([128, 512], F32R, tag="dred")
                        nc.gpsimd.partition_all_reduce(
                            red[:], dacc[:], 128, bass_isa.ReduceOp.add
                        )
                        nc.scalar.activation(red[:], red[:], AF.Ln)
                        rsb = p2tmp.tile([128, 512], F32R, tag="rsb")
                        nc.scalar.activation(rsb[:], red[:], AF.Exp, scale=-1.0)
                        out_t = p2out.tile([128, 512], F32R, tag="outT")
                        nc.vector.tensor_mul(out_t[:], ps_o[:], rsb[:])
                        out_tiles.append(out_t)
                    if prev_out is not None:
                        emit_wo(prev_out, prev_cs)
                    prev_out, prev_cs = out_tiles, cs
                emit_wo(prev_out, prev_cs)
            p2w_stack.close()
            persist_stack.close()
            p1_stack.close()

    nc.compile()
    return nc


def host_prep(inputs, S=S_FULL):
    """Build the 8 per-core input maps from the full problem inputs."""
    x = np.ascontiguousarray(np.asarray(inputs["x"], np.float32))
    cosT = np.ascontiguousarray(np.asarray(inputs["rope_cos"], np.float32).T)
    sinT = np.ascontiguousarray(np.asarray(inputs["rope_sin"], np.float32).T)
    c4 = np.ascontiguousarray(np.concatenate([cosT, cosT, cosT, cosT], 0))
    s4 = np.ascontiguousarray(np.concatenate([-sinT, sinT, -sinT, sinT], 0))
    qw = np.asarray(inputs["q_norm_w"], np.float32)
    kvw = np.asarray(inputs["kv_norm_w"], np.float32)
    W_uq = np.asarray(inputs["W_uq"], np.float32) * qw[:, None]
    W_qr = np.asarray(inputs["W_qr"], np.float32) * qw[:, None]
    W_uk = np.asarray(inputs["W_uk"], np.float32) * kvw[:, None]
    W_kr = np.asarray(inputs["W_kr"], np.float32) * kvw[:, None]
    W_uv = np.asarray(inputs["W_uv"], np.float32) * kvw[:, None]
    W_o = np.asarray(inputs["W_o"], np.float32)
    W_dq = np.ascontiguousarray(np.asarray(inputs["W_dq"], np.float32))
    W_dkv = np.ascontiguousarray(np.asarray(inputs["W_dkv"], np.float32))

    cgrid = np.arange(896)[None, :] - 384
    igrid = np.arange(128)[:, None]
    mask_big = np.where(cgrid >= igrid, 0.0, MASK_NEG).astype(np.float32)
    ident = np.eye(128, dtype=np.float32)

    allgather = bool(int(os.environ.get("MLA_ALLGATHER", "0")))
    in_maps = []
    for c in range(NCORES):
        b, g = c // 4, c % 4
        hs = slice(4 * g * DN, 4 * (g + 1) * DN)
        hr = slice(4 * g * DR, 4 * (g + 1) * DR)
        xT_c = x[b].T[:, 512 * g:512 * (g + 1)] if allgather else x[b].T
        in_maps.append(dict(
            xT=np.ascontiguousarray(xT_c),
            W_dq=W_dq, W_dkv=W_dkv,
            Wuq=np.ascontiguousarray(W_uq[:, hs]),
            Wqr=np.ascontiguousarray(W_qr[:, hr]),
            Wuk=np.ascontiguousarray(W_uk[:, hs]),
            Wkr=np.ascontiguousarray(W_kr[:, hr]),
            Wuv=np.ascontiguousarray(W_uv[:, hs]),
            Wo=np.ascontiguousarray(W_o[512 * g:512 * (g + 1), :]),
            c4=c4, s4=s4, mask_big=mask_big, ident=ident,
            ones_col=np.ones((128, 1), np.float32),
        ))
    return in_maps


_NC_CACHE = {}


def kernel(**inputs) -> np.ndarray:
    S = np.asarray(inputs["x"]).shape[1]
    if S not in _NC_CACHE:
        _NC_CACHE[S] = build_nc(S)
    nc = _NC_CACHE[S]
    in_maps = host_prep(inputs, S)
    trace = bool(os.environ.get("MLA_TRACE"))
    res = run_bass_kernel_spmd(
        nc, in_maps, core_ids=list(range(NCORES)), trace=trace
    )
    if trace:
        print(f"HW exec time: {res.exec_time_ns} ns")
        print(f"trace: {res.instructions_and_trace[1] if res.instructions_and_trace else None}")
    y = np.empty((B, S, D), np.float32)
    for b in range(B):
        acc = res.results[4 * b]["yT"].astype(np.float32).copy()
        for g in range(1, 4):
            acc += res.results[4 * b + g]["yT"]
        y[b] = acc.T
    return y



# revision 44
# speedup vs baseline: 1.5397x; 1.5397x over previous
"""MultiHeadLatentAttention (MLA) Trainium2 Bass kernel, v3.

Problem: B=2, S=2048, D=2048, H=16 heads, d_nope=128, d_rope=64, d_head=128,
q_latent=768, kv_latent=512. Causal attention, rmsnorm'd latents, half-dim RoPE.

Sharding (8 cores): core c handles batch b=c//4 and head group g=c%4 (4 heads).
W_uq/W_qr/W_uk/W_kr/W_uv are column-sharded by head; W_o row-sharded; the 4
partial outputs per batch are summed on the host.

Down-projection sharding (v3): sequence block 0 (columns 0:512) of both
latents is computed replicated on every core; blocks 1-3 are sharded within
each batch group -- core g computes the 128-column quarter
[512b + 128g, 512b + 128(g+1)) of every block b in {1,2,3} (the kv quarters as
one 384-wide section, ditto q), and four bf16 AllGathers (kv, then q blocks
1,2,3) distribute them. Each gather lands well before its consumer: the kv
gather hides under ~45us of gather-independent PE work (replicated block-0
latents, P1 of block 0, q-block-0 projections AND q-block-0's entire
attention, which only needs block-0 k/v), and the q-block gathers hide under
earlier blocks' attention.

Everything is bf16 except the PSUM accumulators, softmax statistics, and rope
tables (matmul cost in TRN2's model is per output column, dtype-independent
at >=256-wide moving operands, so bf16 costs nothing on the PE and halves
DMA/SBUF). Measured end-to-end relative error vs the fp32 reference ~5e-3
(gate: 2e-2).

Scheduling notes (the cost model's engine queues are in-order, so emission
order and tc.tile_wait_until pins matter):
  - rmsnorm tails are sqrt (ACT) + reciprocal (DVE) + a ones-row PE-matmul
    partition broadcast -- only the exp activation table is ever loaded
    (plus sqrt's, once), vs 49 Ln<->Exp table reloads in the baseline.
  - Collectives are pinned to dispatch at their input-ready times; an early
    Pool dispatch would hold the Pool sequencer through the input wait and
    wedge the norm broadcasts behind it.
  - The gather-gated latent-assembly DMAs sit on the SP queue where their
    long semaphore waits only delay y writes that happen much later.
  - W_o is resident in SBUF (bf16); y is emitted two D-tiles per [128,1024]
    PSUM tile borrowed from the scores pool, with PSUM->SBUF staging copies
    alternating between ACT and DVE.
  - Squaring a PSUM tile against itself (DVE dual-read of the same PSUM
    bank) is rejected by the BIR verifier; the rmsnorm squares the copied
    bf16 raws instead.
"""
import math
import os
from contextlib import ExitStack

import numpy as np

import concourse.bass as bass
import concourse.bass_isa as bass_isa
import concourse.bacc as bacc
import concourse.mybir as mybir
import concourse.tile as tile
from concourse.bass_utils import run_bass_kernel_spmd

F32 = mybir.dt.float32
F32R = mybir.dt.float32r
BF16 = mybir.dt.bfloat16
AF = mybir.ActivationFunctionType

B, S_FULL, D = 2, 2048, 2048
H, DN, DR, DH = 16, 128, 64, 128
QL, KVL = 768, 512
EPS = 1e-6
SCALE = 1.0 / math.sqrt(DH)
MASK_NEG = -1e6
NCORES = 8
NKT = D // 128          # 16 contraction tiles over D
NLQ = QL // 128         # 6
NLKV = KVL // 128       # 4
NDT = D // 128          # 16 output D tiles


def _rope_apply(nc, pool, ps, c4s, s4s, out_ap):
    """Half-dim rope on a pair tile [128, 512] (h_even x1|x2 | h_odd x1|x2).

    out = ps * c4 + shuf(ps) * s4,  shuf swaps the 32-blocks within each 64.
    ps is PSUM; out_ap is SBUF.
    """
    shuf = pool.tile([128, 512], F32, tag="rope_shuf")
    nc.scalar.copy(shuf[0:32, :], ps[32:64, :])
    nc.vector.tensor_copy(shuf[32:64, :], ps[0:32, :])
    nc.scalar.copy(shuf[64:96, :], ps[96:128, :])
    nc.vector.tensor_copy(shuf[96:128, :], ps[64:96, :])
    t1 = pool.tile([128, 512], F32, tag="rope_t1")
    nc.vector.tensor_mul(t1[:], ps[:], c4s)
    nc.vector.tensor_mul(out_ap, shuf[:], s4s)
    nc.vector.tensor_add(out_ap, out_ap, t1[:])


PHASE_MARKS = {}


def build_nc(S=S_FULL):
    assert S % 512 == 0
    n_sb = S // 512
    n_st = S // 128
    PHASE_MARKS.clear()

    nc = bacc.Bacc("TRN2", target_bir_lowering=False, debug=False,
                   num_devices=NCORES)

    xb0_d = nc.dram_tensor("xb0", [D, 512], BF16, kind="ExternalInput")
    xq_d = nc.dram_tensor("xq", [D, 384], BF16, kind="ExternalInput")
    wdq_d = nc.dram_tensor("W_dq", [D, QL], BF16, kind="ExternalInput")
    wdkv_d = nc.dram_tensor("W_dkv", [D, KVL], BF16, kind="ExternalInput")
    wuq_d = nc.dram_tensor("Wuq", [QL, 512], BF16, kind="ExternalInput")
    wqr_d = nc.dram_tensor("Wqr", [QL, 256], BF16, kind="ExternalInput")
    wuk_d = nc.dram_tensor("Wuk", [KVL, 512], BF16, kind="ExternalInput")
    wkr_d = nc.dram_tensor("Wkr", [KVL, 256], BF16, kind="ExternalInput")
    wuv_d = nc.dram_tensor("Wuv", [KVL, 512], BF16, kind="ExternalInput")
    wo_d = nc.dram_tensor("Wo", [512, D], BF16, kind="ExternalInput")
    c4_d = nc.dram_tensor("c4", [128, S], F32, kind="ExternalInput")
    s4_d = nc.dram_tensor("s4", [128, S], F32, kind="ExternalInput")
    mask_d = nc.dram_tensor("mask_big", [128, 896], F32R, kind="ExternalInput")
    id_d = nc.dram_tensor("ident", [128, 128], F32R, kind="ExternalInput")
    ones_d = nc.dram_tensor("ones_col", [128, 1], F32R, kind="ExternalInput")
    onesr_d = nc.dram_tensor("ones_row", [1, 128], F32R, kind="ExternalInput")
    yT_d = nc.dram_tensor("yT", [D, S], F32, kind="ExternalOutput")

    def col3(dram_ap, p=128):
        # [R, C] dram slice -> [128, R//128, C] tiled AP
        return dram_ap.rearrange("(t p) c -> p t c", p=p)

    with tile.TileContext(nc) as tc:
        with (
            tc.tile_pool(name="const", bufs=1) as constp,
            tc.tile_pool(name="ps_mm", bufs=2, space="PSUM") as ps_mm,
            tc.tile_pool(name="ps_o", bufs=2, space="PSUM") as ps_op,
            tc.tile_pool(name="dram", bufs=1, space="DRAM") as dramp,
        ):
            kv_in = dramp.tile([KVL, 384], BF16, name="kv_in")
            kv_out = dramp.tile([4 * KVL, 384], BF16, name="kv_out")
            q_ins = [dramp.tile([QL, 128], BF16, name=f"q_in{b}")
                     for b in (1, 2, 3)]
            q_outs = [dramp.tile([4 * QL, 128], BF16, name=f"q_out{b}")
                      for b in (1, 2, 3)]

            def alt_ps(i):
                if i % 2 == 0:
                    return ps_mm.tile([128, 512], F32, tag="mm", name="ps")
                return ps_op.tile([128, 512], F32, tag="pv", name="ps")

            mask_t = constp.tile([128, 896], F32R)
            id_t = constp.tile([128, 128], F32R)
            ones_t = constp.tile([128, 1], F32R)
            ones_row = constp.tile([1, 128], F32R)
            eps_t = constp.tile([1, 1], F32)

            # Persistent q latents (bf16, all blocks) live here until P2.
            qlat_stack = ExitStack()
            qlatp = qlat_stack.enter_context(tc.tile_pool(name="qlat", bufs=1))
            # P1-critical tiles reserved BEFORE the big P0 pools so their DMAs
            # don't anti-depend on P0's SBUF space (the gathered kv latents and
            # k/v up-projection weights gate P1's first matmuls).
            early_stack = ExitStack()
            p1w = early_stack.enter_context(tc.tile_pool(name="p1w", bufs=1))
            p1lat = early_stack.enter_context(
                tc.tile_pool(name="p1lat", bufs=4))
            wuk_t = p1w.tile([128, NLKV, 512], BF16)
            wkr_t = p1w.tile([128, NLKV, 256], BF16)
            wuv_t = p1w.tile([128, NLKV, 512], BF16)
            # q up-projection weights, rope tables, and the small P2 working
            # pools also live from before P0 so the q-block-0 projections can
            # be hoisted between P0 and P1.
            p2w_stack = ExitStack()
            p2w = p2w_stack.enter_context(tc.tile_pool(name="p2w", bufs=1))
            p2q = p2w_stack.enter_context(tc.tile_pool(name="p2q", bufs=8))
            p2qr = p2w_stack.enter_context(tc.tile_pool(name="p2qr", bufs=4))
            p2tmp = p2w_stack.enter_context(tc.tile_pool(name="p2tmp", bufs=2))
            pcs = p2w_stack.enter_context(tc.tile_pool(name="pcs", bufs=4))
            wuq_t = p2w.tile([128, NLQ, 512], BF16)
            wqr_t = p2w.tile([128, NLQ, 256], BF16)

            # ---------------- P0: down-projections + rmsnorm ----------------
            # Block 0 (columns 0:512) of both latents is computed replicated
            # on every core (it is needed before any gather can land). Blocks
            # 1-3 are sharded: each core computes a 384-column strip of the
            # kv latents and three 128-column quarters of the q latents, and
            # a cascade of four AllGathers (kv first, then q blocks 1,2,3)
            # distributes them -- each lands well before its consumer.
            PHASE_MARKS["P0"] = nc.next_id()
            with (
                tc.tile_pool(name="p0w", bufs=1) as p0w,
                tc.tile_pool(name="p0x", bufs=1) as p0x,
                tc.tile_pool(name="p0kv", bufs=1) as p0kv,
                tc.tile_pool(name="p0tmp", bufs=2) as p0tmp,
                tc.tile_pool(name="ps_den", bufs=2, space="PSUM") as ps_denp,
                tc.tile_pool(name="ps_p0", bufs=2, space="PSUM") as ps_p0,
            ):
                def alt3_ps(i):
                    if i % 3 == 2:
                        return ps_p0.tile([128, 512], F32, tag="p0", name="ps")
                    return alt_ps(i % 3)

                wdkv_t = p0w.tile([128, NKT, KVL], BF16)
                wdq_t = p0w.tile([128, NKT, QL], BF16)
                xq_t = p0x.tile([128, NKT, 384], BF16)
                xb0_t = p0x.tile([128, NKT, 512], BF16)
                nc.vector.memset(eps_t[:], EPS)
                # 4-kt group loads, wdkv/xq interleaved so the first kv
                # accumulation chain starts after ~2 transfers.
                for g2 in range(2):
                    ks2 = slice(2 * g2, 2 * (g2 + 1))
                    nc.sync.dma_start(wdkv_t[:, ks2, :],
                                      col3(wdkv_d[:])[:, ks2, :])
                    nc.sync.dma_start(xq_t[:, ks2, :],
                                      col3(xq_d[:])[:, ks2, :])
                nc.sync.dma_start(ones_t[:], ones_d[:])
                nc.sync.dma_start(ones_row[:], onesr_d[:])
                for g4 in range(1, 4):
                    ks4 = slice(4 * g4, 4 * (g4 + 1))
                    nc.sync.dma_start(wdkv_t[:, ks4, :],
                                      col3(wdkv_d[:])[:, ks4, :])
                    nc.sync.dma_start(xq_t[:, ks4, :],
                                      col3(xq_d[:])[:, ks4, :])

                def rms_factor(ps_ss, nl, pool, w, psi):
                    """[128,w] bcast rsqrt(mean+eps) from the sumsq PSUM row.

                    The partition broadcast is a ones-row PE matmul: putting
                    it on the Pool queue creates ordering hazards with the
                    collectives' long sequencer holds, which cost far more
                    than the occasional short PE stall this causes.
                    """
                    srow = pool.tile([1, 512], F32, tag="srow")
                    nc.scalar.activation(
                        srow[:, 0:w], ps_ss[:, 0:w], AF.Sqrt,
                        scale=1.0 / (128 * nl), bias=eps_t[:],
                    )
                    rrow = pool.tile([1, 512], F32R, tag="rrow")
                    with nc.allow_low_precision(reason="rsqrt row, not accum"):
                        nc.vector.reciprocal(rrow[:, 0:w], srow[:, 0:w])
                    rsb = alt3_ps(psi)
                    nc.tensor.matmul(rsb[:, 0:w], ones_row[:], rrow[:, 0:w],
                                     start=True, stop=True)
                    return rsb

                def down_chain(w_t, xs, nl, out_t, w, psi=0):
                    """nl accumulation chains over NKT kt, w-wide moving;
                    writes bf16 raw latents into out_t. Returns a closure
                    that finishes the rmsnorm -- emitted after the NEXT
                    section's chains so the sqrt/recip/broadcast tail never
                    starves the PE queue."""
                    ps_ss = ps_denp.tile([1, 512], F32, tag="den")
                    for lt in range(nl):
                        ps = alt3_ps(psi + lt)
                        for kt in range(NKT):
                            nc.tensor.matmul(
                                ps[:, 0:w],
                                w_t[:, kt, 128 * lt:128 * (lt + 1)],
                                xs[kt],
                                start=(kt == 0), stop=(kt == NKT - 1),
                            )
                        nc.scalar.copy(out_t[:, lt, :], ps[:, 0:w])
                        # NB: squaring the PSUM tile directly (ps x ps) is
                        # rejected by the BIR verifier (same-PSUM dual read);
                        # square the copied bf16 raw instead.
                        sq = p0tmp.tile([128, 512], F32R, tag="sq")
                        nc.vector.tensor_mul(sq[:, 0:w], out_t[:, lt, :],
                                             out_t[:, lt, :])
                        nc.tensor.matmul(
                            ps_ss[:, 0:w], ones_t[:], sq[:, 0:w],
                            start=(lt == 0), stop=(lt == nl - 1),
                        )

                    def finish():
                        rsb = rms_factor(ps_ss, nl, p0tmp, w, psi + nl)
                        for lt in range(nl):
                            nc.vector.tensor_mul(out_t[:, lt, :],
                                                 out_t[:, lt, :],
                                                 rsb[:, 0:w])
                    return finish

                for lc in range(3):
                    ls = slice(256 * lc, 256 * (lc + 1))
                    nc.sync.dma_start(wdq_t[:, :, ls], col3(wdq_d[:, ls]))

                # --- kv strip (one 128-col quarter of each of blocks 1-3,
                # concatenated), then q quarters, then the replicated block 0
                # of both latents.  Norm tails run on ACT/DVE/Pool only, so
                # the next section's chains keep the PE busy through them.
                rawkvq = p0kv.tile([128, NLKV, 384], BF16)
                down_chain(wdkv_t, [xq_t[:, kt, :] for kt in range(NKT)],
                           NLKV, rawkvq, 384)()
                nc.sync.dma_start(col3(kv_in[:]), rawkvq[:])
                # Pin each collective's scheduled slot to its input-ready
                # time: an early Pool dispatch would hold the Pool sequencer
                # through the input wait, wedging the later norm broadcasts
                # that the input transitively needs.
                with tc.tile_wait_until(0.026):
                    nc.gpsimd.collective_compute(
                        "AllGather", mybir.AluOpType.bypass,
                        replica_groups=[[0, 1, 2, 3], [4, 5, 6, 7]],
                        ins=[kv_in[:]], outs=[kv_out[:]],
                    )
                for g4 in range(4):
                    ks4 = slice(4 * g4, 4 * (g4 + 1))
                    nc.sync.dma_start(xb0_t[:, ks4, :],
                                      col3(xb0_d[:])[:, ks4, :])
                rawq = p0kv.tile([128, NLQ, 384], BF16, tag="rawqq",
                                 name="rawqq")
                down_chain(wdq_t, [xq_t[:, kt, :] for kt in range(NKT)],
                           NLQ, rawq, 384, psi=1)()
                for bi, b in enumerate((1, 2, 3)):
                    nc.sync.dma_start(col3(q_ins[bi][:]),
                                      rawq[:, :, 128 * bi:128 * (bi + 1)])
                    with tc.tile_wait_until(0.034 + 0.008 * bi):
                        nc.gpsimd.collective_compute(
                            "AllGather", mybir.AluOpType.bypass,
                            replica_groups=[[0, 1, 2, 3], [4, 5, 6, 7]],
                            ins=[q_ins[bi][:]], outs=[q_outs[bi][:]],
                        )
                nc.sync.dma_start(wuk_t[:], col3(wuk_d[:]))
                nc.sync.dma_start(wkr_t[:], col3(wkr_d[:]))
                nc.sync.dma_start(wuv_t[:], col3(wuv_d[:]))
                nc.sync.dma_start(wuq_t[:], col3(wuq_d[:]))
                nc.sync.dma_start(wqr_t[:], col3(wqr_d[:]))
                c4_ts, s4_ts = [], []
                for sb in range(n_sb):
                    cs = slice(512 * sb, 512 * (sb + 1))
                    c4s = pcs.tile([128, 512], F32, tag="c4")
                    s4s = pcs.tile([128, 512], F32, tag="s4")
                    nc.sync.dma_start(c4s[:], c4_d[:, cs])
                    nc.sync.dma_start(s4s[:], s4_d[:, cs])
                    c4_ts.append(c4s)
                    s4_ts.append(s4s)
                kvl0 = p1lat.tile([128, NLKV, 512], BF16, tag="kvl",
                                  name="kvl0")
                down_chain(wdkv_t, [xb0_t[:, kt, :] for kt in range(NKT)],
                           NLKV, kvl0, 512)()
                qlat0 = qlatp.tile([128, NLQ, 512], BF16, tag="qlat0",
                                   name="qlat0")
                down_chain(wdq_t, [xb0_t[:, kt, :] for kt in range(NKT)],
                           NLQ, qlat0, 512)()

            nc.sync.dma_start(mask_t[:], mask_d[:])
            nc.sync.dma_start(id_t[:], id_d[:])
            # Persistent attention state + W_o load, ahead of the gather-gated
            # assembly DMAs so the W_o transfer isn't stuck behind their long
            # SP-queue waits.
            persist_stack = ExitStack()
            persist = persist_stack.enter_context(
                tc.tile_pool(name="persist", bufs=1)
            )
            kTn_t = persist.tile([128, 4, S], BF16)    # nope k^T per head
            kTr_t = persist.tile([128, 2, S], BF16)    # rope k^T per pair
            v_t = persist.tile([128, n_st, 512], BF16)  # v natural
            wo_t = persist.tile([128, 4, D], BF16)      # W_o resident
            nc.sync.dma_start(wo_t[:], col3(wo_d[:]))

            # Assemble kv latent blocks 1-3 from the gathered quarter strips
            # (rank r's strip columns [128*bi : 128*(bi+1)) hold block bi+1's
            # s-columns [512*(bi+1)+128r, +128)). On the SP queue: everything
            # behind them there (W_o load, P2 y writes) is needed much later
            # than the gather lands.
            kvl_ts = [kvl0]
            with tc.tile_wait_until(0.082):
                for bi, b in enumerate((1, 2, 3)):
                    kvl = p1lat.tile([128, NLKV, 512], BF16, tag="kvl",
                                     name=f"kvl{b}")
                    for r in range(4):
                        nc.sync.dma_start(
                            kvl[:, :, 128 * r:128 * (r + 1)],
                            col3(kv_out[KVL * r:KVL * (r + 1),
                                        128 * bi:128 * (bi + 1)]))
                    kvl_ts.append(kvl)

            # Assemble q latent blocks 1-3 (rank r holds the b-th quarter's
            # columns [128r:128(r+1)) of block b). On the Pool queue,
            # emitted lazily (block b during q-block b-2's attention) so
            # each DMA's gather wait sits between the all_reduce batches it
            # cannot delay.
            def qlat_asm(bi, eng):
                b = bi + 1
                qlat = qlatp.tile([128, NLQ, 512], BF16, tag=f"qlat{b}",
                                  name=f"qlat{b}")
                for r in range(4):
                    eng.dma_start(
                        qlat[:, :, 128 * r:128 * (r + 1)],
                        col3(q_outs[bi][QL * r:QL * (r + 1), :]))
                return qlat

            # qlat1 on the Pool queue, scheduled early so its collective wait
            # resolves to "gather #2 done" rather than a later one; qlat2 on
            # SP where the long wait blocks only y writes that come later;
            # qlat3 lazily on Pool during q-block 2.
            qlat_ts = [qlat0, None, None, None]
            with tc.tile_wait_until(0.0826):
                qlat_ts[1] = qlat_asm(0, nc.sync)
            with tc.tile_wait_until(0.0827):
                qlat_ts[2] = qlat_asm(1, nc.sync)

            # ---------------- P1: k/v up-projections ----------------
            PHASE_MARKS["P1"] = nc.next_id()

            def p1_sb(sb, ps_fn):
                cs = slice(512 * sb, 512 * (sb + 1))
                kvl = kvl_ts[sb]
                c4s, s4s = c4_ts[sb], s4_ts[sb]

                def kv_mov(kl):
                    return kvl[:, kl, :]

                def kv_stat(kl, stl):
                    return kvl[:, kl, 128 * stl:128 * (stl + 1)]

                for h in range(4):
                    ps = ps_fn(h)
                    for kl in range(NLKV):
                        nc.tensor.matmul(
                            ps[:], wuk_t[:, kl, 128 * h:128 * (h + 1)],
                            kv_mov(kl),
                            start=(kl == 0), stop=(kl == NLKV - 1),
                        )
                    nc.scalar.copy(kTn_t[:, h, cs], ps[:])
                for pr in range(2):
                    ps = ps_fn(pr)
                    for kl in range(NLKV):
                        nc.tensor.matmul(
                            ps[:], wkr_t[:, kl, 128 * pr:128 * (pr + 1)],
                            kv_mov(kl),
                            start=(kl == 0), stop=(kl == NLKV - 1),
                        )
                    _rope_apply(nc, p2tmp, ps, c4s[:], s4s[:],
                                kTr_t[:, pr, cs])
                for stl in range(4):
                    st = 4 * sb + stl
                    ps = ps_fn(stl)
                    for kl in range(NLKV):
                        nc.tensor.matmul(
                            ps[:],
                            kv_stat(kl, stl),
                            wuv_t[:, kl, :],
                            start=(kl == 0), stop=(kl == NLKV - 1),
                        )
                    nc.scalar.copy(v_t[:, st, :], ps[:])

            def qproj(qlat, c4s, s4s, raw):
                """q up-projections for one block: (qr_tiles, qn_tiles).
                raw=True reads the rank-major gathered layout."""
                def q_mov(ql):
                    return qlat[:, ql, :]

                qr_tiles = []
                for pr in range(2):
                    ps = ps_mm.tile([128, 512], F32, tag="mm", name="ps")
                    for ql in range(NLQ):
                        nc.tensor.matmul(
                            ps[:], wqr_t[:, ql, 128 * pr:128 * (pr + 1)],
                            q_mov(ql),
                            start=(ql == 0), stop=(ql == NLQ - 1),
                        )
                    qr = p2qr.tile([128, 512], BF16, tag="qr")
                    _rope_apply(nc, p2tmp, ps, c4s[:], s4s[:], qr[:])
                    qr_tiles.append(qr)
                qn_tiles = []
                for h in range(4):
                    ps = ps_mm.tile([128, 512], F32, tag="mm", name="ps")
                    for ql in range(NLQ):
                        nc.tensor.matmul(
                            ps[:], wuq_t[:, ql, 128 * h:128 * (h + 1)],
                            q_mov(ql),
                            start=(ql == 0), stop=(ql == NLQ - 1),
                        )
                    qn = p2q.tile([128, 512], BF16, tag="qn", name="qn")
                    nc.scalar.copy(qn[:], ps[:])
                    qn_tiles.append(qn)
                return qr_tiles, qn_tiles

            # Attention working pools live in the persist stack: q-block 0's
            # whole attention is hoisted before P1 of blocks 1-3 (it depends
            # only on block-0 k/v), so the kv gather has ~45us of PE work to
            # hide behind.
            p2exp = persist_stack.enter_context(
                tc.tile_pool(name="p2exp", bufs=3))
            ps_sp = persist_stack.enter_context(
                tc.tile_pool(name="ps_s", bufs=2, space="PSUM"))
            p2acc = persist_stack.enter_context(
                tc.tile_pool(name="p2acc", bufs=2))
            p2acc1 = persist_stack.enter_context(
                tc.tile_pool(name="p2acc1", bufs=1))
            p2out = persist_stack.enter_context(
                tc.tile_pool(name="p2out", bufs=9))
            p2y = persist_stack.enter_context(
                tc.tile_pool(name="p2y", bufs=2))
            if True:
                def emit_wo(out_tiles, cs):
                    # Two dt blocks per [128,1024] PSUM tile (borrowed from
                    # the scores pool, idle between pair bursts) so the
                    # matmul->copy->DMA ring is not throttled by ps_mm's two
                    # banks; copies alternate ACT/DVE.
                    for dt2 in range(NDT // 2):
                        ps_y = ps_sp.tile([128, 1024], F32, tag="scores",
                                          name="ps_y")
                        for j in (0, 1):
                            dt = 2 * dt2 + j
                            for h in range(4):
                                nc.tensor.matmul(
                                    ps_y[:, 512 * j:512 * (j + 1)],
                                    wo_t[:, h, 128 * dt:128 * (dt + 1)],
                                    out_tiles[h][:],
                                    start=(h == 0), stop=(h == 3),
                                )
                        ystage = p2y.tile([128, 2, 512], F32, tag="y")
                        if dt2 % 2 == 0:
                            nc.scalar.copy(ystage[:], ps_y[:])
                        else:
                            nc.vector.tensor_copy(ystage[:], ps_y[:])
                        nc.sync.dma_start(
                            yT_d[256 * dt2:256 * (dt2 + 1), cs].rearrange(
                                "(t p) c -> p t c", p=128),
                            ystage[:],
                        )

                def qb_attn(qb, qr_tiles, qn_tiles, prev_out, prev_cs):
                    cs = slice(512 * qb, 512 * (qb + 1))
                    out_tiles = []
                    for h in range(4):
                        if h == 1 and prev_out is not None:
                            emit_wo(prev_out, prev_cs)
                        qn = qn_tiles[h]
                        qr = qr_tiles[h // 2]
                        pb = 64 * (h % 2)
                        nkt = 4 * (qb + 1)
                        ps_o = ps_op.tile([128, 512], F32, tag="pv")
                        # softmax denominator accumulates on the DVE; exp
                        # batches two score tiles per ACT op.
                        dacc = p2acc.tile([128, 512], F32R, tag="dacc")

                        def emit_pv(exp_pair, pk, npair, ps_o=ps_o, h=h):
                            for j in (0, 1):
                                kt = 2 * pk + j
                                nc.tensor.matmul(
                                    ps_o[:],
                                    v_t[:, kt, 128 * h:128 * (h + 1)],
                                    exp_pair[:, 512 * j:512 * (j + 1)],
                                    start=(kt == 0), stop=(kt == 2 * npair - 1),
                                )

                        npair = nkt // 2
                        pend = []   # (exp pair tile, pk) one pair behind
                        for pk in range(npair):
                            ps_s = ps_sp.tile([128, 1024], F32, tag="scores")
                            for j in (0, 1):
                                kt = 2 * pk + j
                                ks = slice(128 * kt, 128 * (kt + 1))
                                delta = 128 * kt - 512 * qb
                                diag = delta >= 0
                                half = ps_s[:, 512 * j:512 * (j + 1)]
                                nc.tensor.matmul(
                                    half, kTn_t[:, h, ks], qn[:],
                                    start=True, stop=False,
                                )
                                nc.tensor.matmul(
                                    half,
                                    kTr_t[pb:pb + 64, h // 2, ks],
                                    qr[pb:pb + 64, :],
                                    start=False, stop=not diag,
                                )
                                if diag:
                                    nc.tensor.matmul(
                                        half, id_t[:],
                                        mask_t[:, 384 - delta:896 - delta],
                                        start=False, stop=True,
                                    )
                            exp_t = p2exp.tile([128, 1024], BF16, tag="exp")
                            nc.scalar.activation(
                                exp_t[:], ps_s[:], AF.Exp, scale=SCALE
                            )
                            if pk == 0:
                                nc.vector.tensor_copy(dacc[:], exp_t[:, 0:512])
                            else:
                                nc.vector.tensor_add(
                                    dacc[:], dacc[:], exp_t[:, 0:512])
                            nc.vector.tensor_add(
                                dacc[:], dacc[:], exp_t[:, 512:1024])
                            pend.append((exp_t, pk))
                            if len(pend) > 1:
                                emit_pv(*pend.pop(0), npair)
                        for e in pend:
                            emit_pv(*e, npair)
                        red = p2acc1.tile([128, 512], F32R, tag="dred")
                        nc.gpsimd.partition_all_reduce(
                            red[:], dacc[:], 128, bass_isa.ReduceOp.add
                        )
                        rsb = p2tmp.tile([128, 512], F32, tag="rsb")
                        nc.vector.reciprocal(rsb[:], red[:])
                        out_t = p2out.tile([128, 512], BF16, tag="outT")
                        nc.vector.tensor_mul(out_t[:], ps_o[:], rsb[:])
                        out_tiles.append(out_t)
                    return out_tiles, cs

            # P1 for block 0 (local kv latents), q-block-0 projections, and
            # q-block-0 attention: all gather-independent PE work filling the
            # window until the kv AllGather lands.
            p1_sb(0, alt_ps)
            qp0 = qproj(qlat_ts[0], c4_ts[0], s4_ts[0], raw=False)
            prev = qb_attn(0, qp0[0], qp0[1], None, None)

            # ---------------- P1: blocks 1-3 ----------------
            with tc.tile_wait_until(0.085):
                for sb in range(1, n_sb):
                    p1_sb(sb, alt_ps)

            # ---------------- P2: attention + W_o, blocks 1-3 ----------------
            PHASE_MARKS["P2"] = nc.next_id()
            if True:
                for qb in range(1, n_sb):
                    if qb == 2:
                        with tc.tile_wait_until(0.19):
                            qlat_ts[3] = qlat_asm(2, nc.gpsimd)
                    qr_tiles, qn_tiles = qproj(
                        qlat_ts[qb], c4_ts[qb], s4_ts[qb], raw=True)
                    prev = qb_attn(qb, qr_tiles, qn_tiles, *prev)
                emit_wo(*prev)
            persist_stack.close()
            p2w_stack.close()
            early_stack.close()
            qlat_stack.close()

    nc.compile()
    return nc


def host_prep(inputs, S=S_FULL):
    """Build the 8 per-core input maps from the full problem inputs."""
    import ml_dtypes
    bf16 = ml_dtypes.bfloat16

    x = np.ascontiguousarray(np.asarray(inputs["x"], np.float32))
    cosT = np.ascontiguousarray(np.asarray(inputs["rope_cos"], np.float32).T)
    sinT = np.ascontiguousarray(np.asarray(inputs["rope_sin"], np.float32).T)
    c4 = np.ascontiguousarray(np.concatenate([cosT, cosT, cosT, cosT], 0))
    s4 = np.ascontiguousarray(np.concatenate([-sinT, sinT, -sinT, sinT], 0))
    qw = np.asarray(inputs["q_norm_w"], np.float32)
    kvw = np.asarray(inputs["kv_norm_w"], np.float32)
    W_uq = np.asarray(inputs["W_uq"], np.float32) * qw[:, None]
    W_qr = np.asarray(inputs["W_qr"], np.float32) * qw[:, None]
    W_uk = np.asarray(inputs["W_uk"], np.float32) * kvw[:, None]
    W_kr = np.asarray(inputs["W_kr"], np.float32) * kvw[:, None]
    W_uv = np.asarray(inputs["W_uv"], np.float32) * kvw[:, None]
    W_o = np.asarray(inputs["W_o"], np.float32)
    W_dq = np.ascontiguousarray(np.asarray(inputs["W_dq"], np.float32))
    W_dkv = np.ascontiguousarray(np.asarray(inputs["W_dkv"], np.float32))

    cgrid = np.arange(896)[None, :] - 384
    igrid = np.arange(128)[:, None]
    mask_big = np.where(cgrid >= igrid, 0.0, MASK_NEG).astype(np.float32)
    ident = np.eye(128, dtype=np.float32)

    def b16(a):
        return np.ascontiguousarray(a.astype(bf16))

    in_maps = []
    for c in range(NCORES):
        b, g = c // 4, c % 4
        hs = slice(4 * g * DN, 4 * (g + 1) * DN)
        hr = slice(4 * g * DR, 4 * (g + 1) * DR)
        xT = x[b].T
        in_maps.append(dict(
            xb0=b16(xT[:, 0:512]),
            xq=b16(np.concatenate(
                [xT[:, 512 * bb + 128 * g:512 * bb + 128 * (g + 1)]
                 for bb in (1, 2, 3)], axis=1)),
            W_dq=b16(W_dq), W_dkv=b16(W_dkv),
            Wuq=b16(W_uq[:, hs]),
            Wqr=b16(W_qr[:, hr]),
            Wuk=b16(W_uk[:, hs]),
            Wkr=b16(W_kr[:, hr]),
            Wuv=b16(W_uv[:, hs]),
            Wo=b16(W_o[512 * g:512 * (g + 1), :]),
            c4=c4, s4=s4, mask_big=mask_big, ident=ident,
            ones_col=np.ones((128, 1), np.float32),
            ones_row=np.ones((1, 128), np.float32),
        ))
    return in_maps


_NC_CACHE = {}


def kernel(**inputs) -> np.ndarray:
    S = np.asarray(inputs["x"]).shape[1]
    if S not in _NC_CACHE:
        _NC_CACHE[S] = build_nc(S)
    nc = _NC_CACHE[S]
    in_maps = host_prep(inputs, S)
    trace = bool(os.environ.get("MLA_TRACE"))
    res = run_bass_kernel_spmd(
        nc, in_maps, core_ids=list(range(NCORES)), trace=trace
    )
    if trace:
        print(f"HW exec time: {res.exec_time_ns} ns")
    y = np.empty((B, S, D), np.float32)
    for b in range(B):
        acc = res.results[4 * b]["yT"].astype(np.float32).copy()
        for g in range(1, 4):
            acc += res.results[4 * b + g]["yT"]
        y[b] = acc.T
    return y
